# revision 1
# baseline (speedup 1.0000x reference)
"""Causal GQA self-attention (RoPE) Trainium2 Bass kernel, 8-core SPMD.

Sharding: core c -> (b = c//4, g = c%4).  Data-parallel over batch B=2,
tensor-parallel over the 4 KV groups (4 query heads + 1 KV head each).
Each core computes a partial output y_bg = attn_out_g @ Wo[:, g-block].T
for its batch (bf16 partials); the host sums the 4 group partials per
batch in f32 (row-parallel linear unshard).

Per-core device kernel (matmuls bf16, f32 PSUM accumulation):
  xT arrives pre-transposed from host      [d=128, 16, L]
  kT = RoPE(Wk @ xT)  [d, L]   (perm-matmul rotate + cos/sin DVE muls)
  vn = (x @ Wv^T)     [l, hd]  computed directly (xT stationary)
  qT = RoPE(Wq @ xT)  [d, 4, L]
  attention, qi-major: per (qi, h): per 128-key tile: S^T = K^T q on PE,
    exp on ACT (scale folded), causal via column slices + one triangular
    mask mul on diagonal tiles; softmax denominator = DVE column-sum of
    es tiles + one ones-matmul to replicate the partition reduction;
    attn@V accumulated on PE; normalize with DVE reciprocal.
  y-proj interleaved per qi: y[lt] += oT @ WoT, ACT copy to bf16, DMA out.
"""

import math
import sys

import numpy as np

try:
    import concourse.bass as bass  # noqa: F401
except ImportError:  # pragma: no cover
    sys.path.insert(0, "/opt/trn_rl_repo")
    import concourse.bass as bass  # noqa: F401

import ml_dtypes

import concourse.bacc as bacc
import concourse.mybir as mybir
import concourse.tile as tile
from concourse.bass_utils import run_bass_kernel_spmd

BF16 = ml_dtypes.bfloat16
F32 = np.float32

B, L, D = 2, 2048, 2048
HD = 128          # head dim
NHL = 4           # query heads per core (one KV group)
P = 128
NDT = D // P      # 16 d-tiles
NKT = L // P      # 16 key tiles
NLC = L // 512    # 4 512-wide l chunks
SM_SCALE = 1.0 / math.sqrt(HD)

_BF = mybir.dt.bfloat16
_F32 = mybir.dt.float32

DEBUG_DUMP = False   # add qT/kT/vn/oT DRAM dumps for numeric bisection


def build_nc():
    nc = bacc.Bacc("TRN2", target_bir_lowering=False, debug=False,
                   enable_asserts=False)

    xt_d = nc.dram_tensor("xT", [P, NDT, L], _BF, kind="ExternalInput").ap()
    wq_d = nc.dram_tensor("wq", [P, NHL, NDT, 128], _BF,
                          kind="ExternalInput").ap()
    wk_d = nc.dram_tensor("wk", [P, NDT, 128], _BF, kind="ExternalInput").ap()
    wv_d = nc.dram_tensor("wv", [P, NDT, 128], _BF, kind="ExternalInput").ap()
    wo_d = nc.dram_tensor("wo", [P, NHL, L], _BF, kind="ExternalInput").ap()
    cos_d = nc.dram_tensor("cosT", [P, L], _BF, kind="ExternalInput").ap()
    sin_d = nc.dram_tensor("sinT", [P, L], _BF, kind="ExternalInput").ap()
    perm_d = nc.dram_tensor("perm", [P, P], _BF, kind="ExternalInput").ap()
    tri_d = nc.dram_tensor("tri", [P, P], _BF, kind="ExternalInput").ap()
    y_d = nc.dram_tensor("y", [L, D], _BF, kind="ExternalOutput").ap()
    dbg = None
    if DEBUG_DUMP:
        dbg = {
            "qTd": nc.dram_tensor("qTd", [P, NHL, L], _BF,
                                  kind="ExternalOutput").ap(),
            "kTd": nc.dram_tensor("kTd", [P, L], _BF,
                                  kind="ExternalOutput").ap(),
            "vnd": nc.dram_tensor("vnd", [P, NKT, 128], _BF,
                                  kind="ExternalOutput").ap(),
            "oTd": nc.dram_tensor("oTd", [P, NHL, L], _BF,
                                  kind="ExternalOutput").ap(),
        }

    with tile.TileContext(nc) as tc:
        _body(nc, tc, xt_d, wq_d, wk_d, wv_d, wo_d, cos_d, sin_d,
              perm_d, tri_d, y_d, dbg)
    nc.compile()
    return nc


def _body(nc, tc, xt_d, wq_d, wk_d, wv_d, wo_d, cos_d, sin_d,
          perm_d, tri_d, y_d, dbg=None):
    from contextlib import ExitStack
    ctx = ExitStack()
    with ctx:
        pp = ctx.enter_context(tc.tile_pool(name="persist", bufs=1))
        wsb = ctx.enter_context(tc.tile_pool(name="wsb", bufs=2))

        xT = pp.tile([P, NDT, L], _BF, tag="xT")
        wq_sb = pp.tile([P, NHL, NDT, 128], _BF, tag="wq")
        wk_sb = pp.tile([P, NDT, 128], _BF, tag="wk")
        wv_sb = pp.tile([P, NDT, 128], _BF, tag="wv")
        wo_sb = pp.tile([P, NHL, L], _BF, tag="wo")
        cos_sb = pp.tile([P, L], _BF, tag="cos")
        sin_sb = pp.tile([P, L], _BF, tag="sin")
        perm_sb = pp.tile([P, P], _BF, tag="perm")
        tri_sb = pp.tile([P, P], _BF, tag="tri")
        qT = pp.tile([P, NHL, L], _BF, tag="qT")
        kT = pp.tile([P, L], _BF, tag="kT")
        vn = pp.tile([P, NKT, 128], _BF, tag="vn")
        oT = pp.tile([P, NHL, L], _BF, tag="oT")
        ones_sb = pp.tile([P, P], _BF, tag="ones")
        nc.vector.memset(ones_sb[:], 1.0)

        # The DMA transfer stage is one serial resource, so ordering is
        # everything: the small K/V weights first, then the xT stream that
        # paces pass 1, then the weights needed progressively later (wq
        # per head, so Q-head-0 can start right after the xT stream ends).
        nc.sync.dma_start(wk_sb[:, 0, :], wk_d[:, 0, :])
        nc.sync.dma_start(xT[:, 0, 0:512], xt_d[:, 0, 0:512])
        nc.sync.dma_start(xT[:, 0, 512:L], xt_d[:, 0, 512:L])
        nc.sync.dma_start(wk_sb[:, 1:NDT, :], wk_d[:, 1:NDT, :])
        nc.sync.dma_start(wv_sb[:], wv_d[:])
        for dti in range(1, NDT):
            nc.sync.dma_start(xT[:, dti, :], xt_d[:, dti, :])
        for h in range(NHL):
            nc.sync.dma_start(wq_sb[:, h, :, :], wq_d[:, h, :, :])
        nc.sync.dma_start(cos_sb[:], cos_d[:])
        nc.sync.dma_start(sin_sb[:], sin_d[:])
        nc.sync.dma_start(perm_sb[:], perm_d[:])
        nc.sync.dma_start(tri_sb[:], tri_d[:])
        nc.sync.dma_start(wo_sb[:], wo_d[:])

        def rope_stage(prj, nm, tag="qs", bufs=6):
            """ACT-copy the f32 PSUM projection into SBUF bf16."""
            qs = wsb.tile([P, 512], _BF, tag=tag, bufs=bufs, name=f"qs_{nm}")
            nc.scalar.copy(qs[:], prj[:])
            return qs

        # One PSUM pool with 8 explicitly-managed bank tags: every reuse is
        # a fine-grained per-bank WAR dependency instead of a pool-boundary
        # convoy.
        psum = ctx.enter_context(tc.tile_pool(name="psum", bufs=1,
                                              space="PSUM"))
        _bk = [0]

        def bank(i, nm):
            return psum.tile([P, 512], _F32, tag=f"bk{i}", bufs=1, name=nm)

        def pairt(i, nm):
            # two-bank tile: each [:, j, :] sub-region is exactly one PSUM
            # bank (= one zero region), so per-region start=True is safe,
            # and one ACT instruction can span both banks
            return psum.tile([P, 2, 512], _F32, tag=f"pr{i}", bufs=1, name=nm)

        def rope_tail(qs, dst, lc, nm):
            """dst[:, ls] = qs*cos + (perm@qs)*sin."""
            ls = slice(lc * 512, (lc + 1) * 512)
            qrot = bank(4 + _bk[0] % 4, f"qrot_{nm}")
            _bk[0] += 1
            nc.tensor.matmul(qrot[:], perm_sb[:], qs[:], start=True, stop=True)
            tt = wsb.tile([P, 512], _BF, tag="tt", bufs=6, name=f"tt_{nm}")
            nc.vector.tensor_mul(tt[:], qs[:], cos_sb[:, ls])
            uu = wsb.tile([P, 512], _BF, tag="uu", bufs=2, name=f"uu_{nm}")
            nc.vector.tensor_mul(uu[:], qrot[:], sin_sb[:, ls])
            nc.vector.tensor_add(dst[:, ls], tt[:], uu[:])

        # ---- projections + RoPE
        # Pass 1 computes K (banks 0-3) and V (banks 4-7) together,
        # dti-outer, so PE consumes each xT d-tile right as its DMA lands.
        # Pass 2 (Q heads, lc-blocked so the stage copies stagger) then runs
        # with xT fully resident, rotating through banks 0-3.
        kqs = []
        pk = [pairt(i, f"prk_{i}") for i in range(2)]
        prjs = [pk[lc // 2][:, lc % 2, :] for lc in range(NLC)]
        vps = [bank(4 + s, f"vp_{s}") for s in range(4)]
        for dti in range(NDT):
            for lc in range(NLC):
                nc.tensor.matmul(
                    prjs[lc][:], wk_sb[:, dti, :],
                    xT[:, dti, lc * 512:(lc + 1) * 512],
                    start=(dti == 0), stop=(dti == NDT - 1))
            for sup in range(4):
                for j in range(4):
                    lt = 4 * sup + j
                    # start only on the bank's FIRST matmul: start_tensor_calc
                    # marks the whole 2KB zero-region pending-zero, so a
                    # per-j start would wipe sibling regions' dti=0 data.
                    # Sibling j>0 first-writes land on pending-zero bytes and
                    # overwrite cleanly.
                    nc.tensor.matmul(
                        vps[sup][:, j * 128:(j + 1) * 128],
                        xT[:, dti, lt * P:(lt + 1) * P],
                        wv_sb[:, dti, :],
                        start=(dti == 0 and j == 0), stop=(dti == NDT - 1),
                        skip_group_check=True)
        for lc in range(NLC):
            kqs.append(rope_stage(prjs[lc], f"k{lc}", tag="kqs"))
        for sup in range(4):
            nc.scalar.copy(vn[:, 4 * sup:4 * sup + 4, :],
                           vps[sup][:].rearrange("p (j c) -> p j c", j=4))

        # heads are software-pipelined: head h's rope tails (PE+DVE) are
        # emitted after head h+1's projection matmuls so the ACT stage
        # copies have a full head of slack.  Q3's projection is deferred
        # into the qi=0 attention region, where it is the PE filler for the
        # ACT-gated exp stream (qi=0 has no y-projection work yet).
        pend_tails = None
        for h in range(NHL - 1):
            qss = []
            pq = [pairt(i, f"prq_{h}_{i}") for i in range(2)]
            for lc in range(NLC):
                prjq = pq[lc // 2][:, lc % 2, :]
                for dti in range(NDT):
                    nc.tensor.matmul(
                        prjq[:], wq_sb[:, h, dti, :],
                        xT[:, dti, lc * 512:(lc + 1) * 512],
                        start=(dti == 0), stop=(dti == NDT - 1))
                qss.append(rope_stage(prjq, f"q{h}{lc}"))
            if h == 1:
                # K rope tails: the pass-1 V copies on banks 4-7 have
                # long drained by now
                for lc in range(NLC):
                    rope_tail(kqs[lc], kT, lc, f"k{lc}")
            if pend_tails is not None:
                hp, pqss = pend_tails
                for lc in range(NLC):
                    rope_tail(pqss[lc], qT[:, hp, :], lc, f"q{hp}{lc}")
            pend_tails = (h, qss)
        hp, pqss = pend_tails
        for lc in range(NLC):
            rope_tail(pqss[lc], qT[:, hp, :], lc, f"q{hp}{lc}")

        q3qs = [None] * NLC

        def q3_block(lc):
            # one lc block of the Q3 projection, on a y-projection bank
            prjq = bank(6 + lc % 2, f"prq_3_{lc}")
            for dti in range(NDT):
                nc.tensor.matmul(
                    prjq[:], wq_sb[:, 3, dti, :],
                    xT[:, dti, lc * 512:(lc + 1) * 512],
                    start=(dti == 0), stop=(dti == NDT - 1))
            q3qs[lc] = rope_stage(prjq, f"q3{lc}")

        # ---- attention (qi-major) with y projection groups interleaved
        # into the PE stream so PE stays busy while ACT streams the exps.
        # Banks: sc rotates 0-3 (depth-4 S/exp pipeline), po alternates 4/5,
        # the denominator-replication matmul takes the opposite of po, and
        # the y-projection accumulators alternate 6/7.
        _scb = [0]
        _pyb = [0]

        def yproj_group(lt, mcp):
            # two adjacent output chunks share one wide staging tile and ONE
            # DMA: halves the per-DMA HWDGE/descriptor overhead on the
            # output tail
            ysb = wsb.tile([P, 1024], _BF, tag="ysb", bufs=6,
                           name=f"ysb_{lt}_{mcp}")
            for half in range(2):
                mc = 2 * mcp + half
                py = bank(6 + _pyb[0] % 2, f"py_{lt}_{mc}")
                _pyb[0] += 1
                for h in range(NHL):
                    nc.tensor.matmul(
                        py[:], oT[:, h, lt * P:(lt + 1) * P],
                        wo_sb[:, h, mc * 512:(mc + 1) * 512],
                        start=(h == 0), stop=(h == NHL - 1))
                # GPSIMD cannot read PSUM on HW; alternate ACT/DVE for balance
                if _pyb[0] % 2 == 0:
                    nc.scalar.copy(ysb[:, half * 512:(half + 1) * 512], py[:])
                else:
                    nc.vector.tensor_copy(
                        ysb[:, half * 512:(half + 1) * 512], py[:])
            nc.sync.dma_start(
                y_d[lt * P:(lt + 1) * P, mcp * 1024:(mcp + 1) * 1024],
                ysb[:])

        pend = []        # deque of (lt, mc) groups awaiting emission
        nch = 0
        for qi in (1, 2, 3, 0):
            q0 = qi * 512
            nvis = q0 // P
            nkt = nvis + 4
            for h in range(NHL):
                psum_o = bank(4 + nch % 2, f"po_{h}_{qi}")
                acc = wsb.tile([P, 512], _BF, tag="acc", bufs=3,
                               name=f"acc_{h}_{qi}")
                ess = {}
                # key tiles in units: pairs of fully-visible tiles share one
                # two-bank PSUM tile and ONE exp instruction; the four
                # diagonal tiles are singles.  attn@V runs two units behind
                # S/exp so PE never waits on ACT.
                units = [(kt, kt + 1) for kt in range(0, nkt, 2)]
                nu = len(units)
                for step in range(nu + 2):
                    if step < nu:
                        unit = units[step]
                        ps2 = pairt(step % 2, f"sc_{h}_{qi}_{step}")
                        es = wsb.tile([P, 2, 512], _BF, tag="es", bufs=6,
                                      name=f"es_{h}_{qi}_{step}")
                        for j, kt in enumerate(unit):
                            off = max(0, (kt - nvis) * P)
                            nc.tensor.matmul(
                                ps2[:, j, off:512],
                                kT[:, kt * P:(kt + 1) * P],
                                qT[:, h, q0 + off:q0 + 512],
                                start=True, stop=True, skip_group_check=True)
                        if unit[1] < nvis:
                            # both fully visible: one exp spans both banks
                            nc.scalar.activation(
                                es[:, :, :], ps2[:, :, :],
                                mybir.ActivationFunctionType.Exp,
                                scale=SM_SCALE)
                        else:
                            for j, kt in enumerate(unit):
                                off = max(0, (kt - nvis) * P)
                                nc.scalar.activation(
                                    es[:, j, off:512], ps2[:, j, off:512],
                                    mybir.ActivationFunctionType.Exp,
                                    scale=SM_SCALE)
                                if kt >= nvis:
                                    nc.vector.tensor_mul(
                                        es[:, j, off:off + P],
                                        es[:, j, off:off + P], tri_sb[:])
                        ess[step] = es
                    if step >= 2:
                        unit = units[step - 2]
                        es = ess.pop(step - 2)
                        for j, kt in enumerate(unit):
                            off = max(0, (kt - nvis) * P)
                            cs = slice(off, 512)
                            nc.tensor.matmul(
                                psum_o[:, cs], vn[:, kt, :], es[:, j, cs],
                                start=(kt == 0), stop=(kt == nkt - 1),
                                skip_group_check=True)
                            # softmax denominator: accumulate es across key
                            # tiles (bf16 adds in the 2x DVE mode; the final
                            # partition reduction happens in f32 on the PE)
                            if kt == 0:
                                nc.vector.tensor_copy(acc[:], es[:, 0, :])
                            else:
                                nc.vector.tensor_add(acc[:, cs], acc[:, cs],
                                                     es[:, j, cs])
                # replicate the partition sums via a ones-matmul into the
                # bank po is not currently using
                psum_sum = bank(4 + (nch + 1) % 2, f"ps_{h}_{qi}")
                nc.tensor.matmul(psum_sum[:], ones_sb[:], acc[:],
                                 start=True, stop=True,
                                 skip_group_check=True)
                rec = wsb.tile([P, 512], _F32, tag="rec", bufs=2,
                               name=f"rec_{h}_{qi}")
                nc.vector.reciprocal(rec[:], psum_sum[:])
                nc.vector.tensor_mul(oT[:, h, q0:q0 + 512],
                                     psum_o[:], rec[:])
                nch += 1
                if qi == 1 and not pend and nch <= 4:
                    # qi=0 has no y-projection work yet: the Q3 projection
                    # lc-blocks are the PE filler instead
                    for lc in (((0,), (1,), (2, 3)))[h] if h < 3 else ():
                        q3_block(lc)
                    if h == 2:
                        for lc in range(NLC):
                            rope_tail(q3qs[lc], qT[:, 3, :], lc, f"q3{lc}")
                    continue
                # keep PE fed with previous-chunk y-projection work;
                # later heads get more filler since the ACT exp deficit
                # accumulates across the chunk (some held back for the
                # final drain).
                for _ in range(2):
                    if pend:
                        yproj_group(*pend.pop(0))
            while pend:
                yproj_group(*pend.pop(0))
            pend = [(lt, mcp) for lt in range(4 * qi, 4 * qi + 4)
                    for mcp in range(2)]
        while pend:
            yproj_group(*pend.pop(0))

        if dbg is not None:
            nc.sync.dma_start(dbg["qTd"][:], qT[:])
            nc.sync.dma_start(dbg["kTd"][:], kT[:])
            nc.sync.dma_start(dbg["vnd"][:], vn[:])
            nc.sync.dma_start(dbg["oTd"][:], oT[:])


def host_constants():
    inv = (1.0 / (10000.0 ** (np.arange(0, HD, 2, dtype=np.float32) / HD))
           ).astype(np.float32)
    t = np.arange(L, dtype=np.float32)
    freqs = t[:, None] * inv[None, :]                    # [L, 64]
    emb = np.concatenate([freqs, freqs], axis=-1)        # [L, 128]
    cosT = np.ascontiguousarray(np.cos(emb).T).astype(BF16)
    sinT = np.ascontiguousarray(np.sin(emb).T).astype(BF16)
    perm = np.zeros((P, P), dtype=F32)
    for i in range(64):
        perm[i + 64, i] = -1.0      # qrot[d] = -q[d+64],  d < 64
        perm[i, i + 64] = 1.0       # qrot[d] =  q[d-64],  d >= 64
    tri = (np.arange(P)[:, None] <= np.arange(P)[None, :]).astype(F32)  # k<=q
    return {
        "cosT": cosT, "sinT": sinT,
        "perm": perm.astype(BF16),
        "tri": tri.astype(BF16),
    }


def make_in_map(consts, x, Wq, Wk, Wv, Wo, b, g):
    qs = slice(g * 512, (g + 1) * 512)
    kvs = slice(g * 128, (g + 1) * 128)
    xt = np.ascontiguousarray(
        x[b].T.reshape(NDT, P, L).transpose(1, 0, 2)).astype(BF16)
    # [P, NHL, NDT, 128]: per-head blocks contiguous along (NDT, 128) so the
    # per-head DMA descriptors stay 4KB
    wq = np.ascontiguousarray(
        Wq[qs].T.reshape(NDT, P, NHL, 128).transpose(1, 2, 0, 3)).astype(BF16)
    wk = np.ascontiguousarray(
        Wk[kvs].T.reshape(NDT, P, 128).transpose(1, 0, 2)).astype(BF16)
    wv = np.ascontiguousarray(
        Wv[kvs].T.reshape(NDT, P, 128).transpose(1, 0, 2)).astype(BF16)
    wo = np.ascontiguousarray(
        Wo[:, qs].T.reshape(NHL, P, D).transpose(1, 0, 2)).astype(BF16)
    return {
        "xT": xt,
        "wq": wq, "wk": wk, "wv": wv, "wo": wo,
        **consts,
    }


_NC_CACHE = {}


def get_nc():
    if "nc" not in _NC_CACHE:
        _NC_CACHE["nc"] = build_nc()
    return _NC_CACHE["nc"]


def kernel(x, Wq, Wk, Wv, Wo):
    x = np.asarray(x, dtype=F32)
    Wq = np.asarray(Wq, dtype=F32)
    Wk = np.asarray(Wk, dtype=F32)
    Wv = np.asarray(Wv, dtype=F32)
    Wo = np.asarray(Wo, dtype=F32)
    nc = get_nc()
    consts = host_constants()
    in_maps = [make_in_map(consts, x, Wq, Wk, Wv, Wo, c // 4, c % 4)
               for c in range(8)]
    # warmup launch: the first execution on a freshly-reset device has
    # produced subtly wrong numerics (cold activation tables); discard it.
    run_bass_kernel_spmd(nc, in_maps, list(range(8)))
    res = run_bass_kernel_spmd(nc, in_maps, list(range(8)))
    outs = [r["y"].astype(F32) for r in res.results]
    y = np.stack([sum(outs[0:4]), sum(outs[4:8])], axis=0).astype(F32)
    return y



# revision 34
# speedup vs baseline: 1.1055x; 1.1055x over previous
"""Causal GQA self-attention (RoPE) Trainium2 Bass kernel, 8-core SPMD.

Sharding: core c -> (b = c//4, g = c%4).  Data-parallel over batch B=2,
tensor-parallel over the 4 KV groups (4 query heads + 1 KV head each).
Each core computes a partial output y_bg = attn_out_g @ Wo[:, g-block].T
for its batch (bf16 partials); the host sums the 4 group partials per
batch in f32 (row-parallel linear unshard).

All four linear projections run as fp8(e4m3) DoubleRow matmuls with an
exact hi/lo decomposition: x = xH + xL, W = WH + WL (both fp8, host-
quantized with power-of-2 scales so every product stream shares one
PSUM scale 2^16), and the product is xH@WH + xL@WH + xH@WL (the xL@WL
term is ~0.03% and dropped).  DoubleRow pairs two 128-deep contraction
planes per instruction at 0.5 cycles/column, so the three streams cost
0.75x of the bf16 matmul while keeping bf16-level accuracy.  The 2^-16
descale is folded into the rope cos/sin tables (K/Q), the V stage-copy
scale, and the y-projection copy scale.  Attention itself (S = K^T q,
attn @ V) stays bf16: with a 128-deep contraction DoubleRow cannot beat
one bf16 matmul.

Per-core device kernel (f32 PSUM accumulation):
  pass 1: kT = RoPE(Wk @ x) and vn = x @ Wv^T, dti-pair-outer so PE
    consumes each x d-plane right as its DMA lands.
  pass 2: qT = RoPE(Wq @ x) heads 0-2 (head 3 deferred as PE filler
    into the qi=0 attention region), rope tails software-pipelined.
  attention, qi-major: per (qi, h): per 128-key tile: S^T = K^T q on PE,
    exp on ACT (scale folded), causal via column slices + triangular
    mask mul on diagonal tiles (gpsimd); softmax denominator = DVE/gp
    column-sum of es tiles + one (1/32)-matmul to replicate the
    partition reduction (so rec = 32/den quantizes oT to fp8 hi/lo in
    the same multiply); attn@V accumulated on PE.
  y-proj interleaved per qi as PE filler: py += oH/oL @ WoH/WoL
    (DoubleRow), copy with 2^-16 descale to bf16, DMA out.
"""

import math
import sys

import numpy as np

try:
    import concourse.bass as bass  # noqa: F401
except ImportError:  # pragma: no cover
    sys.path.insert(0, "/opt/trn_rl_repo")
    import concourse.bass as bass  # noqa: F401

import ml_dtypes

import concourse.bacc as bacc
import concourse.mybir as mybir
import concourse.tile as tile
from concourse.bass_utils import run_bass_kernel_spmd

BF16 = ml_dtypes.bfloat16
FP8 = ml_dtypes.float8_e4m3
F32 = np.float32

B, L, D = 2, 2048, 2048
HD = 128          # head dim
NHL = 4           # query heads per core (one KV group)
P = 128
NDT = D // P      # 16 d-planes
NDP = NDT // 2    # 8 d-plane pairs
NKT = L // P      # 16 key tiles
NLC = L // 512    # 4 512-wide l chunks
SM_SCALE = 1.0 / math.sqrt(HD)

SCALE_X = 32.0        # x fp8 pre-scale
SCALE_W = 2048.0      # weight fp8 pre-scale
SCALE_O = 32.0        # oT fp8 pre-scale (applied via rec = 32/den)
DESCALE = 1.0 / (SCALE_X * SCALE_W)    # 2^-16, folded into cos/sin + copies

_BF = mybir.dt.bfloat16
_F8 = mybir.dt.float8e4
_F32 = mybir.dt.float32
DR = mybir.MatmulPerfMode.DoubleRow

DEBUG_DUMP = False   # add qT/kT/vn DRAM dumps for numeric bisection

import os
TRI_GP = os.environ.get("K_TRI_GP", "0") == "1"      # tri mask on gpsimd
ACC_SPLIT = os.environ.get("K_ACC_SPLIT", "0") == "1"  # dual DVE/gp acc chains
OL_GP = os.environ.get("K_OL_GP", "1") == "1"        # oL subtract on gpsimd
OH_GP = os.environ.get("K_OH_GP", "0") == "1"        # oH quantize copy on gpsimd
Y_ACT_EVERY = int(os.environ.get("K_Y_ACT", "2"))    # every Nth y copy on ACT


def build_nc():
    nc = bacc.Bacc("TRN2", target_bir_lowering=False, debug=False,
                   enable_asserts=False)

    xh_d = nc.dram_tensor("xH", [P, NDT, L], _F8, kind="ExternalInput").ap()
    xl_d = nc.dram_tensor("xL", [P, NDT, L], _F8, kind="ExternalInput").ap()
    wqh_d = nc.dram_tensor("wqH", [P, NHL, NDT, 128], _F8,
                           kind="ExternalInput").ap()
    wql_d = nc.dram_tensor("wqL", [P, NHL, NDT, 128], _F8,
                           kind="ExternalInput").ap()
    wkh_d = nc.dram_tensor("wkH", [P, NDT, 128], _F8, kind="ExternalInput").ap()
    wkl_d = nc.dram_tensor("wkL", [P, NDT, 128], _F8, kind="ExternalInput").ap()
    wvh_d = nc.dram_tensor("wvH", [P, NDT, 128], _F8, kind="ExternalInput").ap()
    wvl_d = nc.dram_tensor("wvL", [P, NDT, 128], _F8, kind="ExternalInput").ap()
    woh_d = nc.dram_tensor("woH", [P, NHL, L], _F8, kind="ExternalInput").ap()
    wol_d = nc.dram_tensor("woL", [P, NHL, L], _F8, kind="ExternalInput").ap()
    cos_d = nc.dram_tensor("cosT", [P, L], _BF, kind="ExternalInput").ap()
    sin_d = nc.dram_tensor("sinT", [P, L], _BF, kind="ExternalInput").ap()
    perm_d = nc.dram_tensor("perm", [P, P], _BF, kind="ExternalInput").ap()
    tri_d = nc.dram_tensor("tri", [P, P], _BF, kind="ExternalInput").ap()
    y_d = nc.dram_tensor("y", [L, D], _BF, kind="ExternalOutput").ap()
    dbg = None
    if DEBUG_DUMP:
        dbg = {
            "qTd": nc.dram_tensor("qTd", [P, NHL, L], _BF,
                                  kind="ExternalOutput").ap(),
            "kTd": nc.dram_tensor("kTd", [P, L], _BF,
                                  kind="ExternalOutput").ap(),
            "vnd": nc.dram_tensor("vnd", [P, NKT, 128], _BF,
                                  kind="ExternalOutput").ap(),
        }

    with tile.TileContext(nc) as tc:
        _body(nc, tc, xh_d, xl_d, wqh_d, wql_d, wkh_d, wkl_d, wvh_d, wvl_d,
              woh_d, wol_d, cos_d, sin_d, perm_d, tri_d, y_d, dbg)
    nc.compile()
    return nc


def _body(nc, tc, xh_d, xl_d, wqh_d, wql_d, wkh_d, wkl_d, wvh_d, wvl_d,
          woh_d, wol_d, cos_d, sin_d, perm_d, tri_d, y_d, dbg=None):
    from contextlib import ExitStack
    ctx = ExitStack()
    with ctx:
        pp = ctx.enter_context(tc.tile_pool(name="persist", bufs=1))
        wsb = ctx.enter_context(tc.tile_pool(name="wsb", bufs=2))

        xH = pp.tile([P, NDT, L], _F8, tag="xH")
        xL = pp.tile([P, NDT, L], _F8, tag="xL")
        wqH = pp.tile([P, NHL, NDT, 128], _F8, tag="wqH")
        wqL = pp.tile([P, NHL, NDT, 128], _F8, tag="wqL")
        wkH = pp.tile([P, NDT, 128], _F8, tag="wkH")
        wkL = pp.tile([P, NDT, 128], _F8, tag="wkL")
        wvH = pp.tile([P, NDT, 128], _F8, tag="wvH")
        wvL = pp.tile([P, NDT, 128], _F8, tag="wvL")
        woH = pp.tile([P, NHL, L], _F8, tag="woH")
        woL = pp.tile([P, NHL, L], _F8, tag="woL")
        cos_sb = pp.tile([P, L], _BF, tag="cos")
        sin_sb = pp.tile([P, L], _BF, tag="sin")
        perm_sb = pp.tile([P, P], _BF, tag="perm")
        tri_sb = pp.tile([P, P], _BF, tag="tri")
        qT = pp.tile([P, NHL, L], _BF, tag="qT")
        kT = pp.tile([P, L], _BF, tag="kT")
        vn = pp.tile([P, NKT, 128], _BF, tag="vn")
        oH = pp.tile([P, NHL, L], _F8, tag="oH")
        oL = pp.tile([P, NHL, L], _F8, tag="oL")
        ones_sb = pp.tile([P, P], _BF, tag="ones")
        # 1/SCALE_O so the ones-matmul denominator replication also folds
        # the fp8 oT pre-scale into rec = SCALE_O / den
        nc.vector.memset(ones_sb[:], 1.0 / SCALE_O)

        # The DMA transfer stage is one serial resource, so ordering is
        # everything: the small K/V weights first, then the xH/xL stream
        # that paces pass 1, then the weights needed progressively later.
        # The x stream paces pass 1 (per-dp: xH then xL so main/crossB run
        # while crossA's xL is in flight).  The first pair is column-split
        # so the very first K matmul starts after ~0.7 us of DMA, and the
        # bulk weight transfers are staged between early x pairs so they
        # never block the stream.
        nc.sync.dma_start(wkH[:], wkh_d[:])
        nc.sync.dma_start(xH[:, 0:2, 0:512], xh_d[:, 0:2, 0:512])
        nc.sync.dma_start(wvH[:], wvh_d[:])
        nc.sync.dma_start(xH[:, 0:2, 512:L], xh_d[:, 0:2, 512:L])
        nc.sync.dma_start(wkL[:], wkl_d[:])
        nc.sync.dma_start(wvL[:], wvl_d[:])
        nc.sync.dma_start(xH[:, 2:4, 0:L], xh_d[:, 2:4, 0:L])
        nc.sync.dma_start(xL[:, 0:2, 0:L], xl_d[:, 0:2, 0:L])
        for dp in range(2, NDP):
            d0 = 2 * dp
            nc.sync.dma_start(xH[:, d0:d0 + 2, :], xh_d[:, d0:d0 + 2, :])
            d0 = 2 * dp - 2
            nc.sync.dma_start(xL[:, d0:d0 + 2, :], xl_d[:, d0:d0 + 2, :])
        # wq for the first heads slots in before the xL tail so pass 2 is
        # never blocked on the projection weights
        nc.sync.dma_start(wqH[:, 0, :, :], wqh_d[:, 0, :, :])
        nc.sync.dma_start(wqL[:, 0, :, :], wql_d[:, 0, :, :])
        nc.sync.dma_start(xL[:, NDT - 2:NDT, 0:L], xl_d[:, NDT - 2:NDT, 0:L])
        nc.sync.dma_start(cos_sb[:], cos_d[:])
        nc.sync.dma_start(sin_sb[:], sin_d[:])
        nc.sync.dma_start(perm_sb[:], perm_d[:])
        for h in range(1, NHL):
            nc.sync.dma_start(wqH[:, h, :, :], wqh_d[:, h, :, :])
            nc.sync.dma_start(wqL[:, h, :, :], wql_d[:, h, :, :])
        nc.sync.dma_start(tri_sb[:], tri_d[:])
        nc.sync.dma_start(woH[:], woh_d[:])
        nc.sync.dma_start(woL[:], wol_d[:])

        def rope_stage(prj, nm, tag="qs", bufs=6):
            """ACT-copy the f32 PSUM projection into SBUF bf16 (scaled)."""
            qs = wsb.tile([P, 512], _BF, tag=tag, bufs=bufs, name=f"qs_{nm}")
            nc.scalar.copy(qs[:], prj[:])
            return qs

        # One PSUM pool with 8 explicitly-managed bank tags: every reuse is
        # a fine-grained per-bank WAR dependency instead of a pool-boundary
        # convoy.
        psum = ctx.enter_context(tc.tile_pool(name="psum", bufs=1,
                                              space="PSUM"))
        _bk = [0]

        def bank(i, nm):
            return psum.tile([P, 512], _F32, tag=f"bk{i}", bufs=1, name=nm)

        def pairt(i, nm):
            # two-bank tile: each [:, j, :] sub-region is exactly one PSUM
            # bank (= one zero region), so per-region start=True is safe,
            # and one ACT instruction can span both banks
            return psum.tile([P, 2, 512], _F32, tag=f"pr{i}", bufs=1, name=nm)

        def rope_tail(qs, dst, lc, nm):
            """dst[:, ls] = qs*cosS + rot64(qs)*sinS  (tables hold 2^-16 and
            the sin table is sign-folded for the rotate-half negation).
            The 64-partition rotate runs as two SBUF->SBUF DMAs, keeping PE
            and PSUM out of the rope entirely."""
            ls = slice(lc * 512, (lc + 1) * 512)
            qrot = wsb.tile([P, 512], _BF, tag="qrot", bufs=4,
                            name=f"qrot_{nm}")
            nc.sync.dma_start(qrot[0:64, :], qs[64:128, :])
            nc.sync.dma_start(qrot[64:128, :], qs[0:64, :])
            tt = wsb.tile([P, 512], _BF, tag="tt", bufs=6, name=f"tt_{nm}")
            nc.vector.tensor_mul(tt[:], qs[:], cos_sb[:, ls])
            uu = wsb.tile([P, 512], _BF, tag="uu", bufs=2, name=f"uu_{nm}")
            nc.vector.tensor_mul(uu[:], qrot[:], sin_sb[:, ls])
            nc.vector.tensor_add(dst[:, ls], tt[:], uu[:])

        def kq_unit(prj, wH, wL, cols, dp, first, last, streams=(0, 1, 2)):
            """Three hi/lo DoubleRow streams for one d-plane pair of a K/Q
            projection chunk: (WH,xH) main + (WL,xH) + (WH,xL) crosses.
            `streams` selects a subset so pass 1 can run the xH-gated
            streams (0,1) as one sweep and the xL-gated one (2) later."""
            d0 = 2 * dp
            ops = ((wH, xH), (wL, xH), (wH, xL))
            for i in streams:
                wt, xt = ops[i]
                nc.tensor.matmul(
                    prj[:], wt[:, d0:d0 + 2, :], xt[:, d0:d0 + 2, cols],
                    start=(first and i == 0), stop=(last and i == 2),
                    perf_mode=DR)

        # ---- projections + RoPE
        # Pass 1 computes K (banks 0-3) and V (banks 4-7) together,
        # dp-outer, so PE consumes each x d-pair right as its DMA lands.
        # Pass 2 (Q heads, lc-blocked so the stage copies stagger) then runs
        # with x fully resident, rotating through banks 0-3.
        kqs = []
        pk = [pairt(i, f"prk_{i}") for i in range(2)]
        prjs = [pk[lc // 2][:, lc % 2, :] for lc in range(NLC)]
        vps = [bank(4 + s, f"vp_{s}") for s in range(4)]

        def v_unit(dp, streams):
            d0 = 2 * dp
            vops = ((xH, wvH), (xH, wvL), (xL, wvH))
            for sup in range(4):
                for j in range(4):
                    lt = 4 * sup + j
                    cols = slice(lt * P, (lt + 1) * P)
                    # start only on the bank's FIRST matmul: start_tensor_calc
                    # marks the whole 2KB zero-region pending-zero, so a
                    # per-j start would wipe sibling regions' data.
                    for i in streams:
                        xt, wt = vops[i]
                        nc.tensor.matmul(
                            vps[sup][:, j * 128:(j + 1) * 128],
                            xt[:, d0:d0 + 2, cols], wt[:, d0:d0 + 2, :],
                            start=(dp == 0 and j == 0 and i == 0),
                            stop=(dp == NDP - 1 and i == 2),
                            perf_mode=DR, skip_group_check=True)

        # dp-outer so PE consumes each x d-pair right as its DMA lands.
        # PE is in-order, so emission order mirrors DMA arrival: the
        # xL-gated cross sweep (stream 2) runs one dp BEHIND the xH-gated
        # streams, matching the xH[dp+1]-before-xL[dp] transfer order.
        for dp in range(NDP + 1):
            if dp < NDP:
                for lc in range(NLC):
                    kq_unit(prjs[lc], wkH, wkL,
                            slice(lc * 512, (lc + 1) * 512), dp,
                            dp == 0, False, streams=(0, 1))
                v_unit(dp, (0, 1))
            if dp >= 1:
                dl = dp - 1
                for lc in range(NLC):
                    kq_unit(prjs[lc], wkH, wkL,
                            slice(lc * 512, (lc + 1) * 512), dl,
                            False, dl == NDP - 1, streams=(2,))
                v_unit(dl, (2,))
        for lc in range(NLC):
            kqs.append(rope_stage(prjs[lc], f"k{lc}", tag="kqs"))
        for sup in range(4):
            # V copies apply the fp8 product descale
            nc.scalar.activation(
                vn[:, 4 * sup:4 * sup + 4, :],
                vps[sup][:].rearrange("p (j c) -> p j c", j=4),
                mybir.ActivationFunctionType.Copy, scale=DESCALE)

        # heads are software-pipelined: head h's rope tails (PE+DVE) are
        # emitted after head h+1's projection matmuls so the ACT stage
        # copies have a full head of slack.  Q3's projection is deferred
        # into the qi=0 attention region, where it is the PE filler for the
        # ACT-gated exp stream (qi=0 has no y-projection work yet).
        pend_tails = None
        for h in range(NHL - 1):
            qss = []
            pq = [pairt(i, f"prq_{h}_{i}") for i in range(2)]
            for lc in range(NLC):
                prjq = pq[lc // 2][:, lc % 2, :]
                for dp in range(NDP):
                    kq_unit(prjq, wqH[:, h], wqL[:, h],
                            slice(lc * 512, (lc + 1) * 512), dp,
                            dp == 0, dp == NDP - 1)
                qss.append(rope_stage(prjq, f"q{h}{lc}"))
            if h == 1:
                # K rope tails: the pass-1 V copies on banks 4-7 have
                # long drained by now
                for lc in range(NLC):
                    rope_tail(kqs[lc], kT, lc, f"k{lc}")
            if pend_tails is not None:
                hp, pqss = pend_tails
                for lc in range(NLC):
                    rope_tail(pqss[lc], qT[:, hp, :], lc, f"q{hp}{lc}")
            pend_tails = (h, qss)
        hp, pqss = pend_tails
        for lc in range(NLC):
            rope_tail(pqss[lc], qT[:, hp, :], lc, f"q{hp}{lc}")

        q3qs = [None] * NLC

        def q3_block(lc):
            # one lc block of the Q3 projection, on a y-projection bank
            prjq = bank(6 + lc % 2, f"prq_3_{lc}")
            for dp in range(NDP):
                kq_unit(prjq, wqH[:, 3], wqL[:, 3],
                        slice(lc * 512, (lc + 1) * 512), dp,
                        dp == 0, dp == NDP - 1)
            q3qs[lc] = rope_stage(prjq, f"q3{lc}")

        # ---- attention (qi-major) with y projection groups interleaved
        # into the PE stream so PE stays busy while ACT streams the exps.
        # Banks: sc rotates 0-3 (depth-4 S/exp pipeline), po alternates 4/5,
        # the denominator-replication matmul takes the opposite of po, and
        # the y-projection accumulators alternate 6/7.
        _scb = [0]
        _pyb = [0]
        _yc = [0]

        def yproj_group(lt, mcp, wide=False):
            # two adjacent output chunks share one wide staging tile and ONE
            # DMA: halves the per-DMA HWDGE/descriptor overhead on the
            # output tail.  wide=True (final drain, attention done) rotates
            # over four PSUM banks instead of two so PE doesn't wait on the
            # copy draining the bank two groups back.
            ysb = wsb.tile([P, 1024], _BF, tag="ysb", bufs=5,
                           name=f"ysb_{lt}_{mcp}")
            for half in range(2):
                mc = 2 * mcp + half
                if wide:
                    py = bank(4 + _pyb[0] % 4, f"py_{lt}_{mc}")
                else:
                    py = bank(6 + _pyb[0] % 2, f"py_{lt}_{mc}")
                _pyb[0] += 1
                cols = slice(mc * 512, (mc + 1) * 512)
                qcols = slice(lt * P, (lt + 1) * P)
                for hp in range(2):
                    hs = slice(2 * hp, 2 * hp + 2)
                    for i, (ot, wt) in enumerate(
                            ((oH, woH), (oL, woH), (oH, woL))):
                        nc.tensor.matmul(
                            py[:], ot[:, hs, qcols], wt[:, hs, cols],
                            start=(hp == 0 and i == 0),
                            stop=(hp == 1 and i == 2),
                            perf_mode=DR)
                # y copy applies the fp8 product descale.  GPSIMD cannot
                # read PSUM on HW; alternate ACT/DVE for balance.
                dst = ysb[:, half * 512:(half + 1) * 512]
                _yc[0] += 1
                if Y_ACT_EVERY and _yc[0] % Y_ACT_EVERY == 0:
                    nc.scalar.activation(dst, py[:],
                                         mybir.ActivationFunctionType.Copy,
                                         scale=DESCALE)
                else:
                    nc.vector.tensor_scalar_mul(dst, py[:], DESCALE)
            nc.sync.dma_start(
                y_d[lt * P:(lt + 1) * P, mcp * 1024:(mcp + 1) * 1024],
                ysb[:])

        pend = []        # deque of (lt, mc) groups awaiting emission
        nch = 0
        _acc_i = [0]
        for qi in (1, 2, 3, 0):
            q0 = qi * 512
            nvis = q0 // P
            nkt = nvis + 4
            for h in range(NHL):
                psum_o = bank(4 + nch % 2, f"po_{h}_{qi}")
                acc = wsb.tile([P, 512], _BF, tag="acc", bufs=3,
                               name=f"acc_{h}_{qi}")
                # denominator runs as two independent accumulation chains:
                # gpsimd takes the EARLY units (their es is ready long
                # before the chunk ends, so the slow Pool engine's chain
                # latency is hidden), DVE the late ones; combined with one
                # DVE add at the end.  qi=0 chunks are all-diagonal (no
                # full first tile for the gp chain) and cheap: DVE only.
                use_split = ACC_SPLIT and nvis >= 4
                gp_units = min(3, (nvis + 4) // 4) if use_split else 0
                accp = (wsb.tile([P, 512], _BF, tag="accp", bufs=2,
                                 name=f"accp_{h}_{qi}") if use_split else None)
                first_acc = [True, True]
                ess = {}
                # key tiles in units: pairs of fully-visible tiles share one
                # two-bank PSUM tile and ONE exp instruction; the four
                # diagonal tiles are singles.  attn@V runs two units behind
                # S/exp so PE never waits on ACT.
                units = [(kt, kt + 1) for kt in range(0, nkt, 2)]
                nu = len(units)
                for step in range(nu + 2):
                    if step < nu:
                        unit = units[step]
                        ps2 = pairt(step % 2, f"sc_{h}_{qi}_{step}")
                        es = wsb.tile([P, 2, 512], _BF, tag="es", bufs=6,
                                      name=f"es_{h}_{qi}_{step}")
                        for j, kt in enumerate(unit):
                            off = max(0, (kt - nvis) * P)
                            nc.tensor.matmul(
                                ps2[:, j, off:512],
                                kT[:, kt * P:(kt + 1) * P],
                                qT[:, h, q0 + off:q0 + 512],
                                start=True, stop=True, skip_group_check=True)
                        if unit[1] < nvis:
                            # both fully visible: one exp spans both banks
                            nc.scalar.activation(
                                es[:, :, :], ps2[:, :, :],
                                mybir.ActivationFunctionType.Exp,
                                scale=SM_SCALE)
                        else:
                            for j, kt in enumerate(unit):
                                off = max(0, (kt - nvis) * P)
                                nc.scalar.activation(
                                    es[:, j, off:512], ps2[:, j, off:512],
                                    mybir.ActivationFunctionType.Exp,
                                    scale=SM_SCALE)
                                if kt >= nvis:
                                    # causal mask on the diagonal 128-block
                                    eng = nc.gpsimd if TRI_GP else nc.vector
                                    eng.tensor_mul(
                                        es[:, j, off:off + P],
                                        es[:, j, off:off + P], tri_sb[:])
                        ess[step] = es
                    if step >= 2:
                        cu = step - 2
                        unit = units[cu]
                        es = ess.pop(cu)
                        chain = 1 if cu < gp_units else 0
                        eng = nc.gpsimd if chain else nc.vector
                        tgt = accp if chain else acc
                        for j, kt in enumerate(unit):
                            off = max(0, (kt - nvis) * P)
                            cs = slice(off, 512)
                            nc.tensor.matmul(
                                psum_o[:, cs], vn[:, kt, :], es[:, j, cs],
                                start=(kt == 0), stop=(kt == nkt - 1),
                                skip_group_check=True)
                            # softmax denominator: accumulate es across key
                            # tiles (the final partition reduction happens
                            # in f32 on the PE)
                            if first_acc[chain]:
                                eng.tensor_copy(tgt[:], es[:, j, :])
                                first_acc[chain] = False
                            else:
                                eng.tensor_add(tgt[:, cs], tgt[:, cs],
                                               es[:, j, cs])
                if use_split:
                    nc.vector.tensor_add(acc[:], acc[:], accp[:])
                # replicate the partition sums via a (1/32)-matmul into the
                # bank po is not currently using; rec = 32/den then folds
                # the fp8 oT pre-scale into the normalize multiply
                psum_sum = bank(4 + (nch + 1) % 2, f"ps_{h}_{qi}")
                nc.tensor.matmul(psum_sum[:], ones_sb[:], acc[:],
                                 start=True, stop=True,
                                 skip_group_check=True)
                rec = wsb.tile([P, 512], _F32, tag="rec", bufs=2,
                               name=f"rec_{h}_{qi}")
                nc.vector.reciprocal(rec[:], psum_sum[:])
                # oT quantize chain: u = psum_o * rec  (= oT * 32, bf16),
                # oH = fp8(u), oL = fp8(u - oH)
                u = wsb.tile([P, 512], _BF, tag="u", bufs=3,
                             name=f"u_{h}_{qi}")
                nc.vector.tensor_mul(u[:], psum_o[:], rec[:])
                qs_ = slice(q0, q0 + 512)
                # qi=0 is processed last (its o-chains gate the final y
                # drain) and every h=3 chain gates the next qi's first
                # fillers: keep those on low-latency DVE, gpsimd otherwise
                crit = qi == 0 or h == 3
                (nc.gpsimd if OH_GP and not crit else nc.vector
                 ).tensor_copy(oH[:, h, qs_], u[:])
                (nc.gpsimd if OL_GP and not crit else nc.vector
                 ).tensor_sub(oL[:, h, qs_], u[:], oH[:, h, qs_])
                nch += 1
                if qi == 1 and not pend and nch <= 4:
                    # qi=0 has no y-projection work yet: the Q3 projection
                    # lc-blocks are the PE filler instead
                    for lc in (((0,), (1,), (2, 3)))[h] if h < 3 else ():
                        q3_block(lc)
                    if h == 2:
                        for lc in range(NLC):
                            rope_tail(q3qs[lc], qT[:, 3, :], lc, f"q3{lc}")
                    continue
                # keep PE fed with previous-chunk y-projection work;
                # later heads get more filler since the ACT exp deficit
                # accumulates across the chunk (some held back for the
                # final drain).
                for _ in range(2):
                    if pend:
                        yproj_group(*pend.pop(0))
            while pend:
                yproj_group(*pend.pop(0))
            pend = [(lt, mcp) for lt in range(4 * qi, 4 * qi + 4)
                    for mcp in range(2)]
        _pyb[0] = 0
        while pend:
            yproj_group(*pend.pop(0), wide=True)

        if dbg is not None:
            nc.sync.dma_start(dbg["qTd"][:], qT[:])
            nc.sync.dma_start(dbg["kTd"][:], kT[:])
            nc.sync.dma_start(dbg["vnd"][:], vn[:])


def host_constants():
    inv = (1.0 / (10000.0 ** (np.arange(0, HD, 2, dtype=np.float32) / HD))
           ).astype(np.float32)
    t = np.arange(L, dtype=np.float32)
    freqs = t[:, None] * inv[None, :]                    # [L, 64]
    emb = np.concatenate([freqs, freqs], axis=-1)        # [L, 128]
    # fp8 product descale 2^-16 folded into the rope tables
    cosT = np.ascontiguousarray(np.cos(emb).T * DESCALE).astype(BF16)
    sinT = np.ascontiguousarray(np.sin(emb).T * DESCALE).astype(BF16)
    perm = np.zeros((P, P), dtype=F32)
    for i in range(64):
        perm[i + 64, i] = -1.0      # qrot[d] = -q[d+64],  d < 64
        perm[i, i + 64] = 1.0       # qrot[d] =  q[d-64],  d >= 64
    tri = (np.arange(P)[:, None] <= np.arange(P)[None, :]).astype(F32)  # k<=q
    return {
        "cosT": cosT, "sinT": sinT,
        "perm": perm.astype(BF16),
        "tri": tri.astype(BF16),
    }


def _hilo(a32):
    """fp8 hi/lo split of a pre-scaled f32 array."""
    hi = a32.astype(FP8)
    lo = (a32 - hi.astype(np.float32)).astype(FP8)
    return hi, lo


def make_in_map(consts, x, Wq, Wk, Wv, Wo, b, g):
    qs = slice(g * 512, (g + 1) * 512)
    kvs = slice(g * 128, (g + 1) * 128)
    xt = np.ascontiguousarray(
        x[b].T.reshape(NDT, P, L).transpose(1, 0, 2)) * SCALE_X
    xh, xl = _hilo(xt.astype(np.float32))
    # [P, NHL, NDT, 128]: per-head blocks contiguous along (NDT, 128) so the
    # per-head DMA descriptors stay 4KB
    wq = np.ascontiguousarray(
        Wq[qs].T.reshape(NDT, P, NHL, 128).transpose(1, 2, 0, 3)) * SCALE_W
    wqh, wql = _hilo(wq.astype(np.float32))
    wk = np.ascontiguousarray(
        Wk[kvs].T.reshape(NDT, P, 128).transpose(1, 0, 2)) * SCALE_W
    wkh, wkl = _hilo(wk.astype(np.float32))
    wv = np.ascontiguousarray(
        Wv[kvs].T.reshape(NDT, P, 128).transpose(1, 0, 2)) * SCALE_W
    wvh, wvl = _hilo(wv.astype(np.float32))
    wo = np.ascontiguousarray(
        Wo[:, qs].T.reshape(NHL, P, D).transpose(1, 0, 2)) * SCALE_W
    woh, wol = _hilo(wo.astype(np.float32))
    return {
        "xH": xh, "xL": xl,
        "wqH": wqh, "wqL": wql, "wkH": wkh, "wkL": wkl,
        "wvH": wvh, "wvL": wvl, "woH": woh, "woL": wol,
        **consts,
    }


_NC_CACHE = {}


def get_nc():
    if "nc" not in _NC_CACHE:
        _NC_CACHE["nc"] = build_nc()
    return _NC_CACHE["nc"]


def kernel(x, Wq, Wk, Wv, Wo):
    x = np.asarray(x, dtype=F32)
    Wq = np.asarray(Wq, dtype=F32)
    Wk = np.asarray(Wk, dtype=F32)
    Wv = np.asarray(Wv, dtype=F32)
    Wo = np.asarray(Wo, dtype=F32)
    nc = get_nc()
    consts = host_constants()
    in_maps = [make_in_map(consts, x, Wq, Wk, Wv, Wo, c // 4, c % 4)
               for c in range(8)]
    # warmup launch: the first execution on a freshly-reset device has
    # produced subtly wrong numerics (cold activation tables); discard it.
    run_bass_kernel_spmd(nc, in_maps, list(range(8)))
    res = run_bass_kernel_spmd(nc, in_maps, list(range(8)))
    outs = [r["y"].astype(F32) for r in res.results]
    y = np.stack([sum(outs[0:4]), sum(outs[4:8])], axis=0).astype(F32)
    return y


# revision 49
# speedup vs baseline: 1.1424x; 1.0334x over previous
"""Causal GQA self-attention (RoPE) Trainium2 Bass kernel, 8-core SPMD.

Sharding: core c -> (b = c//4, g = c%4).  Data-parallel over batch B=2,
tensor-parallel over the 4 KV groups (4 query heads + 1 KV head each).
Each core computes a partial output y_bg = attn_out_g @ Wo[:, g-block].T
for its batch (bf16 partials); the host sums the 4 group partials per
batch in f32 (row-parallel linear unshard).

All four linear projections run as fp8(e4m3) DoubleRow matmuls with an
exact hi/lo decomposition: x = xH + xL, W = WH + WL (both fp8, host-
quantized with power-of-2 scales so every product stream shares one
PSUM scale 2^16), and the product is xH@WH + xL@WH + xH@WL (the xL@WL
term is ~0.03% and dropped).  DoubleRow pairs two 128-deep contraction
planes per instruction at 0.5 cycles/column, so the three streams cost
0.75x of the bf16 matmul while keeping bf16-level accuracy.  The 2^-16
descale is folded into the rope cos/sin tables (K/Q), the V stage-copy
scale, and the y-projection copy scale.  Attention itself (S = K^T q,
attn @ V) stays bf16: with a 128-deep contraction DoubleRow cannot beat
one bf16 matmul.

Per-core device kernel (f32 PSUM accumulation):
  pass 1: kT = RoPE(Wk @ x) and vn = x @ Wv^T, dti-pair-outer so PE
    consumes each x d-plane right as its DMA lands.
  pass 2: qT = RoPE(Wq @ x) heads 0-2 (head 3 deferred as PE filler
    into the qi=0 attention region), rope tails software-pipelined.
  attention, qi-major: per (qi, h): per 128-key tile: S^T = K^T q on PE,
    exp on ACT (scale folded), causal via column slices + triangular
    mask mul on diagonal tiles (gpsimd); softmax denominator = DVE/gp
    column-sum of es tiles + one (1/32)-matmul to replicate the
    partition reduction (so rec = 32/den quantizes oT to fp8 hi/lo in
    the same multiply); attn@V accumulated on PE.
  y-proj interleaved per qi as PE filler: py += oH/oL @ WoH/WoL
    (DoubleRow), copy with 2^-16 descale to bf16, DMA out.
"""

import math
import sys

import numpy as np

try:
    import concourse.bass as bass  # noqa: F401
except ImportError:  # pragma: no cover
    sys.path.insert(0, "/opt/trn_rl_repo")
    import concourse.bass as bass  # noqa: F401

import ml_dtypes

import concourse.bacc as bacc
import concourse.mybir as mybir
import concourse.tile as tile
from concourse.bass_utils import run_bass_kernel_spmd

BF16 = ml_dtypes.bfloat16
FP8 = ml_dtypes.float8_e4m3
F32 = np.float32

B, L, D = 2, 2048, 2048
HD = 128          # head dim
NHL = 4           # query heads per core (one KV group)
P = 128
NDT = D // P      # 16 d-planes
NDP = NDT // 2    # 8 d-plane pairs
NKT = L // P      # 16 key tiles
NLC = L // 512    # 4 512-wide l chunks
SM_SCALE = 1.0 / math.sqrt(HD)

SCALE_X = 32.0        # x fp8 pre-scale
SCALE_W = 2048.0      # weight fp8 pre-scale
SCALE_O = 32.0        # oT fp8 pre-scale (applied via rec = 32/den)
DESCALE = 1.0 / (SCALE_X * SCALE_W)    # 2^-16, folded into cos/sin + copies

_BF = mybir.dt.bfloat16
_F8 = mybir.dt.float8e4
_F32 = mybir.dt.float32
DR = mybir.MatmulPerfMode.DoubleRow

DEBUG_DUMP = False   # add qT/kT/vn DRAM dumps for numeric bisection

import os
TRI_GP = os.environ.get("K_TRI_GP", "0") == "1"      # tri mask on gpsimd
ACC_SPLIT = os.environ.get("K_ACC_SPLIT", "0") == "1"  # dual DVE/gp acc chains
OL_GP = os.environ.get("K_OL_GP", "1") == "1"        # oL subtract on gpsimd
OH_GP = os.environ.get("K_OH_GP", "0") == "1"        # oH quantize copy on gpsimd
Y_ACT_EVERY = int(os.environ.get("K_Y_ACT", "2"))    # every Nth y copy on ACT


def build_nc():
    nc = bacc.Bacc("TRN2", target_bir_lowering=False, debug=False,
                   enable_asserts=False)

    xh_d = nc.dram_tensor("xH", [P, NDT, L], _F8, kind="ExternalInput").ap()
    xl_d = nc.dram_tensor("xL", [P, NDT, L], _F8, kind="ExternalInput").ap()
    wqh_d = nc.dram_tensor("wqH", [P, NHL, NDT, 128], _F8,
                           kind="ExternalInput").ap()
    wql_d = nc.dram_tensor("wqL", [P, NHL, NDT, 128], _F8,
                           kind="ExternalInput").ap()
    wkh_d = nc.dram_tensor("wkH", [P, NDT, 128], _F8, kind="ExternalInput").ap()
    wkl_d = nc.dram_tensor("wkL", [P, NDT, 128], _F8, kind="ExternalInput").ap()
    wvh_d = nc.dram_tensor("wvH", [P, NDT, 128], _F8, kind="ExternalInput").ap()
    wvl_d = nc.dram_tensor("wvL", [P, NDT, 128], _F8, kind="ExternalInput").ap()
    woh_d = nc.dram_tensor("woH", [P, NHL, L], _F8, kind="ExternalInput").ap()
    wol_d = nc.dram_tensor("woL", [P, NHL, L], _F8, kind="ExternalInput").ap()
    cos_d = nc.dram_tensor("cosT", [P, L], _BF, kind="ExternalInput").ap()
    sin_d = nc.dram_tensor("sinT", [P, L], _BF, kind="ExternalInput").ap()
    tri_d = nc.dram_tensor("tri", [P, P], _BF, kind="ExternalInput").ap()
    y_d = nc.dram_tensor("y", [L, D], _BF, kind="ExternalOutput").ap()
    dbg = None
    if DEBUG_DUMP:
        dbg = {
            "qTd": nc.dram_tensor("qTd", [P, NHL, L], _BF,
                                  kind="ExternalOutput").ap(),
            "kTd": nc.dram_tensor("kTd", [P, L], _BF,
                                  kind="ExternalOutput").ap(),
            "vnd": nc.dram_tensor("vnd", [P, NKT, 128], _BF,
                                  kind="ExternalOutput").ap(),
        }

    with tile.TileContext(nc) as tc:
        _body(nc, tc, xh_d, xl_d, wqh_d, wql_d, wkh_d, wkl_d, wvh_d, wvl_d,
              woh_d, wol_d, cos_d, sin_d, tri_d, y_d, dbg)
    nc.compile()
    return nc


def _body(nc, tc, xh_d, xl_d, wqh_d, wql_d, wkh_d, wkl_d, wvh_d, wvl_d,
          woh_d, wol_d, cos_d, sin_d, tri_d, y_d, dbg=None):
    from contextlib import ExitStack
    ctx = ExitStack()
    with ctx:
        pp = ctx.enter_context(tc.tile_pool(name="persist", bufs=1))
        wsb = ctx.enter_context(tc.tile_pool(name="wsb", bufs=2))

        xH = pp.tile([P, NDT, L], _F8, tag="xH")
        xL = pp.tile([P, NDT, L], _F8, tag="xL")
        wqH = pp.tile([P, NHL, NDT, 128], _F8, tag="wqH")
        wqL = pp.tile([P, NHL, NDT, 128], _F8, tag="wqL")
        wkH = pp.tile([P, NDT, 128], _F8, tag="wkH")
        wkL = pp.tile([P, NDT, 128], _F8, tag="wkL")
        wvH = pp.tile([P, NDT, 128], _F8, tag="wvH")
        wvL = pp.tile([P, NDT, 128], _F8, tag="wvL")
        woH = pp.tile([P, NHL, L], _F8, tag="woH")
        woL = pp.tile([P, NHL, L], _F8, tag="woL")
        cos_sb = pp.tile([P, L], _BF, tag="cos")
        sin_sb = pp.tile([P, L], _BF, tag="sin")
        tri_sb = pp.tile([P, P], _BF, tag="tri")
        qT = pp.tile([P, NHL, L], _BF, tag="qT")
        kT = pp.tile([P, L], _BF, tag="kT")
        vn = pp.tile([P, NKT, 128], _BF, tag="vn")
        oH = pp.tile([P, NHL, L], _F8, tag="oH")
        oL = pp.tile([P, NHL, L], _F8, tag="oL")
        ones_sb = pp.tile([P, P], _BF, tag="ones")
        # 1/SCALE_O so the ones-matmul denominator replication also folds
        # the fp8 oT pre-scale into rec = SCALE_O / den
        nc.vector.memset(ones_sb[:], 1.0 / SCALE_O)

        # The DMA transfer stage is one serial resource, so ordering is
        # everything: the small K/V weights first, then the xH/xL stream
        # that paces pass 1, then the weights needed progressively later.
        # The x stream paces pass 1 (per-dp: xH then xL so main/crossB run
        # while crossA's xL is in flight).  The first pair is column-split
        # so the very first K matmul starts after ~0.7 us of DMA, and the
        # bulk weight transfers are staged between early x pairs so they
        # never block the stream.
        nc.sync.dma_start(wkH[:, 0:2, :], wkh_d[:, 0:2, :])
        nc.sync.dma_start(xH[:, 0:2, 0:512], xh_d[:, 0:2, 0:512])
        nc.sync.dma_start(wvH[:, 0:2, :], wvh_d[:, 0:2, :])
        nc.sync.dma_start(xH[:, 0:2, 512:L], xh_d[:, 0:2, 512:L])
        nc.sync.dma_start(wkL[:, 0:2, :], wkl_d[:, 0:2, :])
        nc.sync.dma_start(wvL[:, 0:2, :], wvl_d[:, 0:2, :])
        nc.sync.dma_start(wkH[:, 2:NDT, :], wkh_d[:, 2:NDT, :])
        nc.sync.dma_start(wvH[:, 2:NDT, :], wvh_d[:, 2:NDT, :])
        nc.sync.dma_start(wkL[:, 2:NDT, :], wkl_d[:, 2:NDT, :])
        nc.sync.dma_start(wvL[:, 2:NDT, :], wvl_d[:, 2:NDT, :])
        nc.sync.dma_start(xH[:, 2:4, 0:L], xh_d[:, 2:4, 0:L])
        nc.sync.dma_start(xL[:, 0:2, 0:L], xl_d[:, 0:2, 0:L])
        for dp in range(2, NDP):
            d0 = 2 * dp
            nc.sync.dma_start(xH[:, d0:d0 + 2, :], xh_d[:, d0:d0 + 2, :])
            d0 = 2 * dp - 2
            nc.sync.dma_start(xL[:, d0:d0 + 2, :], xl_d[:, d0:d0 + 2, :])
        # Q0's first d-pairs slot in before the final xL pair so pass 2
        # starts the moment pass 1's PE drains
        nc.sync.dma_start(wqH[:, 0, 0:8, :], wqh_d[:, 0, 0:8, :])
        nc.sync.dma_start(wqL[:, 0, 0:8, :], wql_d[:, 0, 0:8, :])
        nc.sync.dma_start(xL[:, NDT - 2:NDT, 0:L], xl_d[:, NDT - 2:NDT, 0:L])
        nc.sync.dma_start(wqH[:, 0, 8:NDT, :], wqh_d[:, 0, 8:NDT, :])
        nc.sync.dma_start(wqL[:, 0, 8:NDT, :], wql_d[:, 0, 8:NDT, :])
        nc.sync.dma_start(cos_sb[:], cos_d[:])
        nc.sync.dma_start(sin_sb[:], sin_d[:])
        for h in range(1, NHL):
            nc.sync.dma_start(wqH[:, h, :, :], wqh_d[:, h, :, :])
            nc.sync.dma_start(wqL[:, h, :, :], wql_d[:, h, :, :])
        nc.sync.dma_start(tri_sb[:], tri_d[:])
        nc.sync.dma_start(woH[:], woh_d[:])
        nc.sync.dma_start(woL[:], wol_d[:])

        def rope_stage(prj, nm, tag="qs", bufs=6, eng=None):
            """Copy the f32 PSUM projection into SBUF bf16 (scaled)."""
            qs = wsb.tile([P, 512], _BF, tag=tag, bufs=bufs, name=f"qs_{nm}")
            if eng is None:
                nc.scalar.copy(qs[:], prj[:])
            else:
                eng.tensor_copy(qs[:], prj[:])
            return qs

        # One PSUM pool with 8 explicitly-managed bank tags: every reuse is
        # a fine-grained per-bank WAR dependency instead of a pool-boundary
        # convoy.
        psum = ctx.enter_context(tc.tile_pool(name="psum", bufs=1,
                                              space="PSUM"))
        _bk = [0]

        def bank(i, nm):
            return psum.tile([P, 512], _F32, tag=f"bk{i}", bufs=1, name=nm)

        def pairt(i, nm):
            # two-bank tile: each [:, j, :] sub-region is exactly one PSUM
            # bank (= one zero region), so per-region start=True is safe,
            # and one ACT instruction can span both banks
            return psum.tile([P, 2, 512], _F32, tag=f"pr{i}", bufs=1, name=nm)

        def rope_tail(qs, dst, lc, nm):
            """dst[:, ls] = qs*cosS + rot64(qs)*sinS  (tables hold 2^-16 and
            the sin table is sign-folded for the rotate-half negation).
            The 64-partition rotate runs as two SBUF->SBUF DMAs, keeping PE
            and PSUM out of the rope entirely."""
            ls = slice(lc * 512, (lc + 1) * 512)
            qrot = wsb.tile([P, 512], _BF, tag="qrot", bufs=2,
                            name=f"qrot_{nm}")
            nc.sync.dma_start(qrot[0:64, :], qs[64:128, :])
            nc.sync.dma_start(qrot[64:128, :], qs[0:64, :])
            tt = wsb.tile([P, 512], _BF, tag="tt", bufs=6, name=f"tt_{nm}")
            nc.vector.tensor_mul(tt[:], qs[:], cos_sb[:, ls])
            uu = wsb.tile([P, 512], _BF, tag="uu", bufs=2, name=f"uu_{nm}")
            nc.vector.tensor_mul(uu[:], qrot[:], sin_sb[:, ls])
            nc.vector.tensor_add(dst[:, ls], tt[:], uu[:])

        def kq_unit(prj, wH, wL, cols, dp, first, last, streams=(0, 1, 2)):
            """Three hi/lo DoubleRow streams for one d-plane pair of a K/Q
            projection chunk: (WH,xH) main + (WL,xH) + (WH,xL) crosses.
            `streams` selects a subset so pass 1 can run the xH-gated
            streams (0,1) as one sweep and the xL-gated one (2) later."""
            d0 = 2 * dp
            ops = ((wH, xH), (wL, xH), (wH, xL))
            for i in streams:
                wt, xt = ops[i]
                nc.tensor.matmul(
                    prj[:], wt[:, d0:d0 + 2, :], xt[:, d0:d0 + 2, cols],
                    start=(first and i == 0), stop=(last and i == 2),
                    perf_mode=DR)

        # ---- projections + RoPE
        # Pass 1 computes K (banks 0-3) and V (banks 4-7) together,
        # dp-outer, so PE consumes each x d-pair right as its DMA lands.
        # Pass 2 (Q heads, lc-blocked so the stage copies stagger) then runs
        # with x fully resident, rotating through banks 0-3.
        kqs = []
        pk = [pairt(i, f"prk_{i}") for i in range(2)]
        prjs = [pk[lc // 2][:, lc % 2, :] for lc in range(NLC)]
        vps = [bank(4 + s, f"vp_{s}") for s in range(4)]

        def v_unit(dp, streams):
            d0 = 2 * dp
            vops = ((xH, wvH), (xH, wvL), (xL, wvH))
            for sup in range(4):
                for j in range(4):
                    lt = 4 * sup + j
                    cols = slice(lt * P, (lt + 1) * P)
                    # start only on the bank's FIRST matmul: start_tensor_calc
                    # marks the whole 2KB zero-region pending-zero, so a
                    # per-j start would wipe sibling regions' data.
                    for i in streams:
                        xt, wt = vops[i]
                        nc.tensor.matmul(
                            vps[sup][:, j * 128:(j + 1) * 128],
                            xt[:, d0:d0 + 2, cols], wt[:, d0:d0 + 2, :],
                            start=(dp == 0 and j == 0 and i == 0),
                            stop=(dp == NDP - 1 and i == 2),
                            perf_mode=DR, skip_group_check=True)

        # dp-outer so PE consumes each x d-pair right as its DMA lands.
        # PE is in-order, so emission order mirrors DMA arrival: the
        # xL-gated cross sweep (stream 2) runs one dp BEHIND the xH-gated
        # streams, matching the xH[dp+1]-before-xL[dp] transfer order.
        for dp in range(NDP + 1):
            if dp < NDP:
                for lc in range(NLC):
                    kq_unit(prjs[lc], wkH, wkL,
                            slice(lc * 512, (lc + 1) * 512), dp,
                            dp == 0, False, streams=(0, 1))
                v_unit(dp, (0, 1))
            if dp >= 1:
                dl = dp - 1
                for lc in range(NLC):
                    kq_unit(prjs[lc], wkH, wkL,
                            slice(lc * 512, (lc + 1) * 512), dl,
                            False, dl == NDP - 1, streams=(2,))
                v_unit(dl, (2,))
        for lc in range(NLC):
            kqs.append(rope_stage(prjs[lc], f"k{lc}", tag="kqs"))
        for sup in range(4):
            # V copies apply the fp8 product descale
            nc.scalar.activation(
                vn[:, 4 * sup:4 * sup + 4, :],
                vps[sup][:].rearrange("p (j c) -> p j c", j=4),
                mybir.ActivationFunctionType.Copy, scale=DESCALE)

        # heads are software-pipelined: head h's rope tails (PE+DVE) are
        # emitted after head h+1's projection matmuls so the ACT stage
        # copies have a full head of slack.  Q3's projection is deferred
        # into the qi=0 attention region, where it is the PE filler for the
        # ACT-gated exp stream (qi=0 has no y-projection work yet).
        pend_tails = None
        for h in range(NHL - 1):
            qss = []
            pq = [pairt(i, f"prq_{h}_{i}") for i in range(2)]
            for lc in range(NLC):
                prjq = pq[lc // 2][:, lc % 2, :]
                for dp in range(NDP):
                    kq_unit(prjq, wqH[:, h], wqL[:, h],
                            slice(lc * 512, (lc + 1) * 512), dp,
                            dp == 0, dp == NDP - 1)
                qss.append(rope_stage(prjq, f"q{h}{lc}"))
            if h == 1:
                # K rope tails: the pass-1 V copies on banks 4-7 have
                # long drained by now
                for lc in range(NLC):
                    rope_tail(kqs[lc], kT, lc, f"k{lc}")
            if pend_tails is not None:
                hp, pqss = pend_tails
                for lc in range(NLC):
                    rope_tail(pqss[lc], qT[:, hp, :], lc, f"q{hp}{lc}")
            pend_tails = (h, qss)
        hp, pqss = pend_tails
        for lc in range(NLC):
            rope_tail(pqss[lc], qT[:, hp, :], lc, f"q{hp}{lc}")

        q3qs = [None] * NLC

        def q3_block(lc):
            # one lc block of the Q3 projection, on a y-projection bank
            prjq = bank(6 + lc % 2, f"prq_3_{lc}")
            for dp in range(NDP):
                kq_unit(prjq, wqH[:, 3], wqL[:, 3],
                        slice(lc * 512, (lc + 1) * 512), dp,
                        dp == 0, dp == NDP - 1)
            q3qs[lc] = rope_stage(prjq, f"q3{lc}")

        # ---- attention (qi-major) with y projection groups interleaved
        # into the PE stream so PE stays busy while ACT streams the exps.
        # Banks: sc rotates 0-3 (depth-4 S/exp pipeline), po alternates 4/5,
        # the denominator-replication matmul takes the opposite of po, and
        # the y-projection accumulators alternate 6/7.
        _scb = [0]
        _pyb = [0]
        _yc = [0]

        def yproj_group(lt, mcp, wide=False, split=False):
            # two adjacent output chunks share one wide staging tile and ONE
            # DMA: halves the per-DMA HWDGE/descriptor overhead on the
            # output tail.  wide=True (final drain, attention done) rotates
            # over four PSUM banks instead of two so PE doesn't wait on the
            # copy draining the bank two groups back.  split=True (last
            # groups) DMAs each half separately with the two copies on
            # different engines, shortening the post-matmul drain.
            ysb = wsb.tile([P, 1024], _BF, tag="ysb", bufs=5,
                           name=f"ysb_{lt}_{mcp}")
            for half in range(2):
                mc = 2 * mcp + half
                if wide:
                    py = bank(4 + _pyb[0] % 4, f"py_{lt}_{mc}")
                else:
                    py = bank(6 + _pyb[0] % 2, f"py_{lt}_{mc}")
                _pyb[0] += 1
                cols = slice(mc * 512, (mc + 1) * 512)
                qcols = slice(lt * P, (lt + 1) * P)
                for hp in range(2):
                    hs = slice(2 * hp, 2 * hp + 2)
                    for i, (ot, wt) in enumerate(
                            ((oH, woH), (oL, woH), (oH, woL))):
                        nc.tensor.matmul(
                            py[:], ot[:, hs, qcols], wt[:, hs, cols],
                            start=(hp == 0 and i == 0),
                            stop=(hp == 1 and i == 2),
                            perf_mode=DR)
                # y copy applies the fp8 product descale.  GPSIMD cannot
                # read PSUM on HW; alternate ACT/DVE for balance.
                dst = ysb[:, half * 512:(half + 1) * 512]
                _yc[0] += 1
                on_act = (half == 0 if split else
                          Y_ACT_EVERY and _yc[0] % Y_ACT_EVERY == 0)
                if on_act:
                    nc.scalar.activation(dst, py[:],
                                         mybir.ActivationFunctionType.Copy,
                                         scale=DESCALE)
                else:
                    nc.vector.tensor_scalar_mul(dst, py[:], DESCALE)
                if split:
                    nc.sync.dma_start(
                        y_d[lt * P:(lt + 1) * P, mc * 512:(mc + 1) * 512],
                        dst)
            if not split:
                nc.sync.dma_start(
                    y_d[lt * P:(lt + 1) * P, mcp * 1024:(mcp + 1) * 1024],
                    ysb[:])

        pend = []        # deque of (lt, mc) groups awaiting emission
        nch = 0
        _acc_i = [0]
        for qi in (1, 2, 3, 0):
            q0 = qi * 512
            nvis = q0 // P
            nkt = nvis + 4
            for h in range(NHL):
                psum_o = bank(4 + nch % 2, f"po_{h}_{qi}")
                acc = wsb.tile([P, 512], _BF, tag="acc", bufs=3,
                               name=f"acc_{h}_{qi}")
                # denominator runs as two independent accumulation chains:
                # gpsimd takes the EARLY units (their es is ready long
                # before the chunk ends, so the slow Pool engine's chain
                # latency is hidden), DVE the late ones; combined with one
                # DVE add at the end.  qi=0 chunks are all-diagonal (no
                # full first tile for the gp chain) and cheap: DVE only.
                use_split = ACC_SPLIT and nvis >= 4
                gp_units = min(3, (nvis + 4) // 4) if use_split else 0
                accp = (wsb.tile([P, 512], _BF, tag="accp", bufs=2,
                                 name=f"accp_{h}_{qi}") if use_split else None)
                first_acc = [True, True]
                ess = {}
                # key tiles in units: pairs of fully-visible tiles share one
                # two-bank PSUM tile and ONE exp instruction; the four
                # diagonal tiles are singles.  attn@V runs two units behind
                # S/exp so PE never waits on ACT.
                units = [(kt, kt + 1) for kt in range(0, nkt, 2)]
                nu = len(units)
                rec = wsb.tile([P, 512], _F32, tag="rec", bufs=2,
                               name=f"rec_{h}_{qi}")
                psum_sum = bank(4 + (nch + 1) % 2, f"ps_{h}_{qi}")
                for step in range(nu + 2):
                    if step < nu:
                        unit = units[step]
                        ps2 = pairt(step % 2, f"sc_{h}_{qi}_{step}")
                        es = wsb.tile([P, 2, 512], _BF, tag="es", bufs=6,
                                      name=f"es_{h}_{qi}_{step}")
                        for j, kt in enumerate(unit):
                            off = max(0, (kt - nvis) * P)
                            nc.tensor.matmul(
                                ps2[:, j, off:512],
                                kT[:, kt * P:(kt + 1) * P],
                                qT[:, h, q0 + off:q0 + 512],
                                start=True, stop=True, skip_group_check=True)
                        if unit[1] < nvis:
                            # both fully visible: one exp spans both banks
                            nc.scalar.activation(
                                es[:, :, :], ps2[:, :, :],
                                mybir.ActivationFunctionType.Exp,
                                scale=SM_SCALE)
                        else:
                            for j, kt in enumerate(unit):
                                off = max(0, (kt - nvis) * P)
                                nc.scalar.activation(
                                    es[:, j, off:512], ps2[:, j, off:512],
                                    mybir.ActivationFunctionType.Exp,
                                    scale=SM_SCALE)
                                if kt >= nvis:
                                    # causal mask on the diagonal 128-block
                                    eng = nc.gpsimd if TRI_GP else nc.vector
                                    eng.tensor_mul(
                                        es[:, j, off:off + P],
                                        es[:, j, off:off + P], tri_sb[:])
                        # softmax denominator accumulates right after the
                        # exp (not with attn@V two steps later) so the
                        # chain completes before the attn@V drain and the
                        # rec/normalize tail shrinks
                        chain = 1 if step < gp_units else 0
                        aeng = nc.gpsimd if chain else nc.vector
                        tgt = accp if chain else acc
                        for j, kt in enumerate(unit):
                            off = max(0, (kt - nvis) * P)
                            cs = slice(off, 512)
                            if first_acc[chain]:
                                aeng.tensor_copy(tgt[:], es[:, j, :])
                                first_acc[chain] = False
                            else:
                                aeng.tensor_add(tgt[:, cs], tgt[:, cs],
                                                es[:, j, cs])
                        if step == nu - 1:
                            if use_split:
                                nc.vector.tensor_add(acc[:], acc[:], accp[:])
                        ess[step] = es
                    if step >= 2:
                        cu = step - 2
                        unit = units[cu]
                        es = ess.pop(cu)
                        for j, kt in enumerate(unit):
                            off = max(0, (kt - nvis) * P)
                            cs = slice(off, 512)
                            nc.tensor.matmul(
                                psum_o[:, cs], vn[:, kt, :], es[:, j, cs],
                                start=(kt == 0), stop=(kt == nkt - 1),
                                skip_group_check=True)
                # replicate the partition sums via a (1/32)-matmul into the
                # bank po is not using; thanks to the early acc the chain
                # is already complete, and rec = 32/den (which also folds
                # the fp8 oT pre-scale) runs with no wait
                nc.tensor.matmul(psum_sum[:], ones_sb[:], acc[:],
                                 start=True, stop=True,
                                 skip_group_check=True)
                nc.vector.reciprocal(rec[:], psum_sum[:])
                # oT quantize chain: u = psum_o * rec  (= oT * 32, bf16),
                # oH = fp8(u), oL = fp8(u - oH)
                u = wsb.tile([P, 512], _BF, tag="u", bufs=3,
                             name=f"u_{h}_{qi}")
                nc.vector.tensor_mul(u[:], psum_o[:], rec[:])
                qs_ = slice(q0, q0 + 512)
                # qi=0 is processed last (its o-chains gate the final y
                # drain) and every h=3 chain gates the next qi's first
                # fillers: keep those on low-latency DVE, gpsimd otherwise
                crit = qi == 0 or h == 3
                (nc.gpsimd if OH_GP and not crit else nc.vector
                 ).tensor_copy(oH[:, h, qs_], u[:])
                (nc.gpsimd if OL_GP and not crit else nc.vector
                 ).tensor_sub(oL[:, h, qs_], u[:], oH[:, h, qs_])
                nch += 1
                if qi == 1 and not pend and nch <= 4:
                    # qi=0 has no y-projection work yet: the Q3 projection
                    # lc-blocks are the PE filler instead
                    for lc in (((0,), (1,), (2, 3)))[h] if h < 3 else ():
                        q3_block(lc)
                    if h == 2:
                        for lc in range(NLC):
                            rope_tail(q3qs[lc], qT[:, 3, :], lc, f"q3{lc}")
                    continue
                # keep PE fed with previous-chunk y-projection work;
                # later heads get more filler since the ACT exp deficit
                # accumulates across the chunk (some held back for the
                # final drain).
                for _ in range(2):
                    if pend:
                        yproj_group(*pend.pop(0))
            while pend:
                yproj_group(*pend.pop(0))
            pend = [(lt, mcp) for lt in range(4 * qi, 4 * qi + 4)
                    for mcp in range(2)]
        _pyb[0] = 0
        while pend:
            yproj_group(*pend.pop(0), wide=True, split=len(pend) < 2)

        if dbg is not None:
            nc.sync.dma_start(dbg["qTd"][:], qT[:])
            nc.sync.dma_start(dbg["kTd"][:], kT[:])
            nc.sync.dma_start(dbg["vnd"][:], vn[:])


def host_constants():
    inv = (1.0 / (10000.0 ** (np.arange(0, HD, 2, dtype=np.float32) / HD))
           ).astype(np.float32)
    t = np.arange(L, dtype=np.float32)
    freqs = t[:, None] * inv[None, :]                    # [L, 64]
    emb = np.concatenate([freqs, freqs], axis=-1)        # [L, 128]
    # fp8 product descale 2^-16 folded into the rope tables; the rotate
    # half's negation (rot(q)[d] = -q[d+64] for d<64) is folded into the
    # sign of the sin table's first half (the kernel's rotate is a plain
    # 64-partition swap via DMA)
    cosT = np.ascontiguousarray(np.cos(emb).T * DESCALE).astype(BF16)
    sinT = np.ascontiguousarray(np.sin(emb).T * DESCALE)
    sinT[0:64] *= -1.0
    tri = (np.arange(P)[:, None] <= np.arange(P)[None, :]).astype(F32)  # k<=q
    return {
        "cosT": cosT, "sinT": sinT.astype(BF16),
        "tri": tri.astype(BF16),
    }


def _hilo(a32):
    """fp8 hi/lo split of a pre-scaled f32 array."""
    hi = a32.astype(FP8)
    lo = (a32 - hi.astype(np.float32)).astype(FP8)
    return hi, lo


def make_in_map(consts, x, Wq, Wk, Wv, Wo, b, g):
    qs = slice(g * 512, (g + 1) * 512)
    kvs = slice(g * 128, (g + 1) * 128)
    xt = np.ascontiguousarray(
        x[b].T.reshape(NDT, P, L).transpose(1, 0, 2)) * SCALE_X
    xh, xl = _hilo(xt.astype(np.float32))
    # [P, NHL, NDT, 128]: per-head blocks contiguous along (NDT, 128) so the
    # per-head DMA descriptors stay 4KB
    wq = np.ascontiguousarray(
        Wq[qs].T.reshape(NDT, P, NHL, 128).transpose(1, 2, 0, 3)) * SCALE_W
    wqh, wql = _hilo(wq.astype(np.float32))
    wk = np.ascontiguousarray(
        Wk[kvs].T.reshape(NDT, P, 128).transpose(1, 0, 2)) * SCALE_W
    wkh, wkl = _hilo(wk.astype(np.float32))
    wv = np.ascontiguousarray(
        Wv[kvs].T.reshape(NDT, P, 128).transpose(1, 0, 2)) * SCALE_W
    wvh, wvl = _hilo(wv.astype(np.float32))
    wo = np.ascontiguousarray(
        Wo[:, qs].T.reshape(NHL, P, D).transpose(1, 0, 2)) * SCALE_W
    woh, wol = _hilo(wo.astype(np.float32))
    return {
        "xH": xh, "xL": xl,
        "wqH": wqh, "wqL": wql, "wkH": wkh, "wkL": wkl,
        "wvH": wvh, "wvL": wvl, "woH": woh, "woL": wol,
        **consts,
    }


_NC_CACHE = {}


def get_nc():
    if "nc" not in _NC_CACHE:
        _NC_CACHE["nc"] = build_nc()
    return _NC_CACHE["nc"]


def kernel(x, Wq, Wk, Wv, Wo):
    x = np.asarray(x, dtype=F32)
    Wq = np.asarray(Wq, dtype=F32)
    Wk = np.asarray(Wk, dtype=F32)
    Wv = np.asarray(Wv, dtype=F32)
    Wo = np.asarray(Wo, dtype=F32)
    nc = get_nc()
    consts = host_constants()
    in_maps = [make_in_map(consts, x, Wq, Wk, Wv, Wo, c // 4, c % 4)
               for c in range(8)]
    # warmup launch: the first execution on a freshly-reset device has
    # produced subtly wrong numerics (cold activation tables); discard it.
    run_bass_kernel_spmd(nc, in_maps, list(range(8)))
    res = run_bass_kernel_spmd(nc, in_maps, list(range(8)))
    outs = [r["y"].astype(F32) for r in res.results]
    y = np.stack([sum(outs[0:4]), sum(outs[4:8])], axis=0).astype(F32)
    return y
